# revision 29
# baseline (speedup 1.0000x reference)
"""BinaryConv (BN + sign-binarize + 3x3 binary conv) on 8 Trainium2 NeuronCores.

Strategy (data-parallel over batch, per sharding hint):
  - Each of the 8 cores gets 4 of the 32 images; weights/gamma/beta replicated.
  - Per-core BN partial stats (mean, mean-square per channel) via bn_stats,
    AllGathered across cores (2KB payload, transposed stat-slot-major so the
    gathered blocks read back as 512B-contiguous runs), then the 8-rank sum +
    transpose back to channel-major happens in ONE fp32 PE matmul against a
    replicated identity: sync-BN exact.
  - Binarize via ScalarE Sign(gamma*x + (beta*std - mean*gamma)) — equivalent
    to the reference BN+sign since std>0 — into a zero-padded per-image
    layout (34-wide rows, both ci-halves stacked) in fp8 e4m3.
  - 3x3 conv = 9 shifted DoubleRow fp8 matmuls (contraction 256 in one pass)
    accumulated in PSUM. +/-1 inputs with fp32 PSUM accumulation are exact
    (integer sums), so the conv matches the fp32 reference bit-for-bit.
  - Weights: sign() on ScalarE, transposed to [ci][2][o] via PE transpose.
  - A dense fp8 warm-up matmul burst gated on the collective readback fills
    the stat-math latency window so the PE clock (HAM) is ramped when the
    conv burst starts; the last image's PSUM drain is split across
    vector/scalar (copies) and sync/gpsimd (stores) to shorten the tail.

Timing notes (measured): the collective cannot complete before ~70-90us from
execution start regardless of trigger time (per-execution CC arming under
this runtime; a lone 2KB AllGather takes the same). Everything before it
(x load, stats, weight prep) overlaps that wall; the post-collective path is
~46us: ~6us stats math + binarize lead-in, ~35us conv (at fp8 DoubleRow
peak), ~4.5us drain. remote_dma paths (SWDGE prepare/trigger, hostgen) hang
or fail to load under this runtime — a CC collective is the only working
cross-core channel.
"""

import numpy as np

import concourse.tile as tile
from concourse import bacc, mybir
from concourse.bass_utils import run_bass_kernel_spmd
from concourse.masks import make_identity

F32 = mybir.dt.float32
BF16 = mybir.dt.bfloat16
FP8 = mybir.dt.float8e4

N_CORES = 8
N = 32            # full batch
NLOC = N // N_CORES  # images per core
C = 256           # channels (in == out)
HW = 32           # spatial
CB = C // 128     # ci partition blocks
OB = C // 128     # o partition blocks
EPS = 1e-5

PADW = HW + 2     # padded row width
IMG_PAD = 1160    # per-image padded buffer (>= 34*34 + 2 margin, mult of 8)
# output row-chunks (r0, r1): each chunk's matmul free dim = (r1-r0)*34 <= 512
CHUNKS = [(0, 11), (11, 22), (22, 32)]
TAPS = [(dy, dx) for dy in range(3) for dx in range(3)]


def _wt_idx(t: int, b: int, o: int) -> int:
    return (t * CB + b) * OB + o


def _build_body(ctx, nc, tc, x_d, g_d, be_d, w_d, y_d, cc_in, cc_out, rsum_d):
    # ---------------- pools ----------------
    const = ctx.enter_context(tc.tile_pool(name="const", bufs=1))
    xin_p = ctx.enter_context(tc.tile_pool(name="xin", bufs=1))
    wpool = ctx.enter_context(tc.tile_pool(name="wpool", bufs=1))
    apool = ctx.enter_context(tc.tile_pool(name="apool", bufs=1))
    stat_p = ctx.enter_context(tc.tile_pool(name="stat", bufs=1))
    out_p = ctx.enter_context(tc.tile_pool(name="outp", bufs=1))
    ps_tr = ctx.enter_context(tc.tile_pool(name="pstr", bufs=2, space="PSUM"))
    ps_acc = ctx.enter_context(tc.tile_pool(name="psacc", bufs=1, space="PSUM"))

    # ---------------- load x (stats-critical) ----------------
    # 16 DMAs (one per ci-block x image x row-half) so all 16 HW DMA queues
    # stream in parallel; per-queue bandwidth was the previous limiter.
    xin = []
    x_dmas = []
    for b in range(CB):
        xb = xin_p.tile([128, NLOC, HW, HW], F32, name=f"xin{b}", tag=f"xin{b}")
        for i in range(NLOC):
            for h in range(2):
                eng = nc.sync if b == 0 else nc.scalar
                x_dmas.append(
                    eng.dma_start(
                        out=xb[:, i, 16 * h : 16 * (h + 1), :],
                        in_=x_d[i, 128 * b : 128 * (b + 1),
                                16 * h : 16 * (h + 1), :],
                    )
                )
        xin.append(xb)

    # ---------------- weight prep (independent of stats) ----------------
    ident = const.tile([128, 128], BF16, name="ident")
    make_identity(nc, ident[:])
    identf = const.tile([128, 128], F32, name="identf")
    make_identity(nc, identf[:])
    # replicated identity for the rank-sum matmul: rsum[(k, s), s'] = (s==s').
    # Built via DRAM bounce — compute engines can't write at partition
    # offset 4, DMAs can. All pre-wall.
    rsum = const.tile([N_CORES * 2 * CB, 2 * CB], F32, name="rsum")

    # ---------------- activation-table preload (scalar, off critical path) --
    # The post-collective math needs Sqrt/Copy tables; loading them here
    # (during the x transfer) removes the 1.3-1.5us ACT_TABLE_LOAD that
    # otherwise sits right after the collective readback.
    eps_t = const.tile([128, 1], F32, name="eps_t")
    nc.vector.memset(eps_t[:], EPS)
    dumm = const.tile([128, 1], F32, name="dumm")
    nc.scalar.activation(
        out=dumm[:], in_=eps_t[:],
        func=mybir.ActivationFunctionType.Sign,
    )
    nc.scalar.activation(
        out=dumm[:], in_=eps_t[:],
        func=mybir.ActivationFunctionType.Sqrt,
        bias=eps_t[:], scale=1.0,
    )
    nc.scalar.mul(out=dumm[:], in_=eps_t[:], mul=eps_t[:])
    nc.scalar.add(out=dumm[:], in_=dumm[:], add=eps_t[:])

    # junk rhs for the PE warm-up burst (contents irrelevant; memset only so
    # the race detector sees initialized data)
    dummy_a = const.tile([128, CB, 374], FP8, name="dummy_a")
    nc.gpsimd.memset(dummy_a[:], 0.0)


    # ---------------- zero only the padding of the activation buffers ------
    # (interior is fully overwritten by binarize; tiny strided memsets keep
    # both DVE and the collective-trigger path free)
    apad = [None] * NLOC
    for i in range(NLOC):
        ap = apool.tile([128, CB, IMG_PAD], FP8, name=f"apad{i}",
                        tag=f"apad{i}")
        nc.gpsimd.memset(ap[:, :, 0:35], 0.0)
        gaps = ap[:, :, 67 : 67 + 34 * HW].rearrange(
            "p b (h w) -> p b h w", w=PADW
        )[:, :, :, 0:2]
        nc.gpsimd.memset(gaps, 0.0)
        nc.gpsimd.memset(ap[:, :, 35 + 34 * HW - 2 : IMG_PAD], 0.0)
        apad[i] = ap

    # ---------------- local BN stats ----------------
    stats_rec = []
    for b in range(CB):
        xb = xin[b]
        rec = stat_p.tile([128, 2 * NLOC, 6], F32, name=f"rec{b}", tag=f"rec{b}")
        for i in range(NLOC):
            for h in range(2):
                nc.vector.bn_stats(
                    out=rec[:, 2 * i + h, :],
                    in_=xb[:, i, 16 * h : 16 * (h + 1), :].rearrange(
                        "p h w -> p (h w)"
                    ),
                )
        stats_rec.append(rec)

    # pack [mean_b, meansq_b] per ci-block into AllReduce payload
    arbuf = stat_p.tile([128, 2 * CB], F32, name="arbuf")
    tmp1 = stat_p.tile([128, 1], F32, name="tmp1")
    for b in range(CB):
        mv = stat_p.tile([128, 2], F32, name=f"mv{b}", tag=f"mv{b}")
        nc.vector.bn_aggr(out=mv[:], in_=stats_rec[b][:])
        nc.vector.tensor_copy(out=arbuf[:, 2 * b : 2 * b + 1], in_=mv[:, 0:1])
        nc.vector.tensor_mul(tmp1[:], mv[:, 0:1], mv[:, 0:1])
        nc.vector.tensor_add(arbuf[:, 2 * b + 1 : 2 * b + 2], mv[:, 1:2], tmp1[:])

    # Bounce the 2KB stats payload TRANSPOSED ([4, 128]: stat-slot major) so
    # each rank's gathered block is contiguous per stat-slot. The readback
    # then moves 512B-granule runs (32 descriptors vs 1024 for the
    # channel-major layout), and the 8-rank sum happens on the PE as an
    # accumulating-matmul transpose — no DVE tree-reduce on the critical
    # path. The transposing bounce write (4B granule, ~1us) is pre-wall.
    nc.gpsimd.dma_start(
        out=cc_in.rearrange("s c -> c s"), in_=arbuf[:]
    )
    # AllGather + local 8-way reduce instead of AllReduce: the gather is a
    # single mesh pass (~9.5us) vs ~12.7us for AllReduce.
    nc.gpsimd.collective_compute(
        "AllGather",
        mybir.AluOpType.bypass,
        replica_groups=[list(range(N_CORES))],
        ins=[cc_in.ap().opt()],
        outs=[cc_out.ap().opt()],
    )
    # ------- weight prep: W DMAs issued on the same HWDGE queues as x but
    # later in program order, so they start after the x issue stream drains —
    # no explicit gating needed. 4 DMAs split across both issue engines.
    wsign = []
    for o in range(OB):
        wraw = wpool.tile([128, C, 3, 3], F32, name=f"wraw{o}", tag=f"wraw{o}")
        for ch in range(2):
            (nc.sync if ch == 0 else nc.scalar).dma_start(
                out=wraw[:, 128 * ch : 128 * (ch + 1)],
                in_=w_d[128 * o : 128 * (o + 1),
                        128 * ch : 128 * (ch + 1), :, :],
            )
        ws = wpool.tile([128, C, 3, 3], BF16, name=f"wsign{o}", tag=f"wsign{o}")
        nc.scalar.activation(
            out=ws[:], in_=wraw[:], func=mybir.ActivationFunctionType.Sign
        )
        wsign.append(ws)

    # build rsum (queued after the x/W issue streams so it never delays them)
    nc.sync.dma_start(out=rsum_d[:, :], in_=identf[0 : 2 * CB, 0 : 2 * CB])
    for k in range(N_CORES):
        nc.scalar.dma_start(
            out=rsum[4 * k : 4 * (k + 1), :], in_=rsum_d[:, :]
        )

    # transposed binarized weights, DoubleRow layout:
    # wT[ci_local, tap*OB+o, ci_half, o_local]  (fp8)
    wT = wpool.tile([128, len(TAPS) * OB, CB, 128], FP8, name="wT")
    for t, (dy, dx) in enumerate(TAPS):
        for b in range(CB):
            for o in range(OB):
                ptr = ps_tr.tile([128, 128], BF16, name="ptr", tag="ptr", bufs=2)
                nc.tensor.transpose(
                    ptr[:], wsign[o][:, 128 * b : 128 * (b + 1), dy, dx], ident[:]
                )
                nc.vector.tensor_copy(
                    out=wT[:, t * OB + o, b, :], in_=ptr[:]
                )

    # readback all 8 ranks' transposed partials as one [32, 128] tile
    # ((rank, slot) on partitions; 512B-contiguous runs, 32 descriptors in a
    # single DMA), then ONE fp32 matmul against a replicated-identity [32, 4]
    # rhs sums the ranks AND restores channel-major in ~0.25us.
    gsall_t = stat_p.tile([N_CORES * 2 * CB, 128], F32, name="gsall_t")
    nc.sync.dma_start(
        out=gsall_t[:, :],
        in_=cc_out[:, :, :].rearrange("k s c -> (k s) c"),
    )
    gs_ps = ps_tr.tile([128, 2 * CB], F32, name="gs_ps", tag="ptr", bufs=2)
    nc.tensor.matmul(
        gs_ps[:], gsall_t[:, :], rsum[:, :], start=True, stop=True
    )
    # stat math reads the PSUM sum directly; the SBUF copy only feeds the PE
    # warm-up trigger (off the critical chain)
    gs = stat_p.tile([128, 2 * CB], F32, name="gs")
    nc.vector.tensor_copy(out=gs[:], in_=gs_ps[:])
    smean = gs_ps[:].rearrange("p (b s) -> p b s", s=2)[:, :, 0]  # [128, CB]
    smsq = gs_ps[:].rearrange("p (b s) -> p b s", s=2)[:, :, 1]

    # per-channel scale/shift computed as wide [128, CB] ops.
    # Since std > 0:  sign((x-mean)*gamma/std + beta)
    #              == sign(gamma*x + (beta*std - mean*gamma))
    # so scale = gamma (known before the AllReduce!) and
    # shift = beta*std - mean*gamma  (no reciprocal needed).
    gam = stat_p.tile([128, CB], F32, name="gam")
    bet = stat_p.tile([128, CB], F32, name="bet")
    for b in range(CB):
        nc.sync.dma_start(out=gam[:, b : b + 1], in_=g_d[128 * b : 128 * (b + 1), :])
        nc.sync.dma_start(out=bet[:, b : b + 1], in_=be_d[128 * b : 128 * (b + 1), :])
    inv = 1.0 / N_CORES
    # PE warm-up trigger: gpsimd (not ACT/DVE, which are on the critical
    # stat-math path) casts the summed stats into the warm-up rhs.
    junk = stat_p.tile([128, 4], BF16, name="junk")
    nc.gpsimd.tensor_copy(out=junk[:], in_=gs[:])
    mean_t = stat_p.tile([128, CB], F32, name="mean_t")
    nc.vector.tensor_scalar_mul(out=mean_t[:], in0=smean, scalar1=inv)
    msqr = stat_p.tile([128, CB], F32, name="msqr")
    nc.vector.tensor_mul(msqr[:], mean_t[:], mean_t[:])
    var_t = stat_p.tile([128, CB], F32, name="var_t")
    # var = (smsq * inv) - mean^2
    nc.vector.scalar_tensor_tensor(
        out=var_t[:],
        in0=smsq,
        scalar=inv,
        in1=msqr[:],
        op0=mybir.AluOpType.mult,
        op1=mybir.AluOpType.subtract,
    )
    # neg_mg = -(mean)*gamma, computed on DVE in parallel with var
    neg_mg = stat_p.tile([128, CB], F32, name="neg_mg")
    nc.vector.scalar_tensor_tensor(
        out=neg_mg[:],
        in0=smean,
        scalar=-inv,
        in1=gam[:],
        op0=mybir.AluOpType.mult,
        op1=mybir.AluOpType.mult,
    )
    # sqrt + shift entirely on ScalarE: binarize follows on the same queue
    # with no cross-engine hop
    std_t = stat_p.tile([128, CB], F32, name="std_t")
    nc.scalar.activation(
        out=std_t[:],
        in_=var_t[:],
        func=mybir.ActivationFunctionType.Sqrt,
        bias=eps_t[:],
        scale=1.0,
    )
    # shift = beta*std + neg_mg on DVE (idle here), freeing the scalar queue
    # so the binarize SIGNs start right after the sqrt
    sh_t = stat_p.tile([128, CB], F32, name="sh_t")
    nc.vector.tensor_mul(sh_t[:], std_t[:], bet[:])
    nc.vector.tensor_add(sh_t[:], sh_t[:], neg_mg[:])
    scale_t = [gam[:, b : b + 1] for b in range(CB)]
    shift_t = [sh_t[:, b : b + 1] for b in range(CB)]

    # warm-up matmuls (results discarded) — dense fp8 DoubleRow bursts sized
    # to fill the stat-math + binarize latency window. Tiny matmuls don't
    # trigger the HAM clock ramp (observed: ramp fires ~3.2us into a dense
    # burst); these do, so the conv starts at full PE clock.
    pw0 = ps_tr.tile([128, 4], F32, name="pw0", tag="ptr", bufs=2)
    nc.tensor.matmul(pw0[:], ident[:], junk[:], start=True, stop=True)
    for k in range(10):
        pw = ps_tr.tile([128, 374], F32, name="pw", tag="ptr", bufs=2)
        nc.tensor.matmul(
            pw[:],
            wT[:, 0, :, :],
            dummy_a[:, :, 0:374],
            start=True,
            stop=True,
            perf_mode=mybir.MatmulPerfMode.DoubleRow,
        )

    # ---------------- binarize into padded layout (fp8, DoubleRow pairs) ----
    # Row-halves so the first conv chunk (rows 0..13) can start as soon as
    # the top halves of both ci-blocks are written.
    for i in range(NLOC):
        for h in range(2):
            for b in range(CB):
                interior = apad[i][:, b, 35 : 35 + 34 * HW].rearrange(
                    "p (h w) -> p h w", w=PADW
                )[:, 16 * h : 16 * (h + 1), 0:HW]
                nc.scalar.activation(
                    out=interior,
                    in_=xin[b][:, i, 16 * h : 16 * (h + 1), :],
                    func=mybir.ActivationFunctionType.Sign,
                    scale=scale_t[b],
                    bias=shift_t[b],
                )

    # ---------------- conv: 9 shifted DoubleRow matmuls, PSUM accumulate ----
    for i in range(NLOC):
        psum = {}
        for o in range(OB):
            for ci, (r0, r1) in enumerate(CHUNKS):
                psum[(o, ci)] = ps_acc.tile(
                    [128, (r1 - r0) * PADW], F32, name=f"acc{o}_{ci}",
                    tag=f"acc{o}_{ci}", bufs=1,
                )
        for t, (dy, dx) in enumerate(TAPS):
            toff = dy * PADW + dx
            first = t == 0
            last = t == len(TAPS) - 1
            # chunk-major so each tap's chunk-0 matmuls (which only need the
            # top half-rows of the binarized image) issue before the chunk-1/2
            # ones that wait on the bottom half-rows.
            for ci, (r0, r1) in enumerate(CHUNKS):
                ncols = (r1 - r0) * PADW
                off = r0 * PADW + toff
                for o in range(OB):
                    nc.tensor.matmul(
                        psum[(o, ci)][:],
                        wT[:, t * OB + o, :, :],
                        apad[i][:, :, off : off + ncols],
                        start=first,
                        stop=last,
                        perf_mode=mybir.MatmulPerfMode.DoubleRow,
                    )
        last_img = i == NLOC - 1
        for o in range(OB):
            osb = out_p.tile([128, HW, HW], F32, name=f"osb{o}", tag=f"osb{o}",
                             bufs=2)
            for ci, (r0, r1) in enumerate(CHUNKS):
                src = psum[(o, ci)][:].rearrange("p (r c) -> p r c", c=PADW)[
                    :, :, 0:HW
                ]
                # last image: binarize is done, so scalar (copies) and gpsimd
                # (DMA issue) are free — split the drain across both pairs of
                # engines to halve the kernel tail.
                if last_img and (o * len(CHUNKS) + ci) % 2 == 1:
                    nc.scalar.activation(
                        out=osb[:, r0:r1, :], in_=src,
                        func=mybir.ActivationFunctionType.Copy,
                    )
                    dma_eng = nc.gpsimd
                else:
                    nc.vector.tensor_copy(out=osb[:, r0:r1, :], in_=src)
                    dma_eng = nc.sync
                # per-chunk output DMA so the store of the final chunks
                # overlaps the remaining matmuls instead of tailing the kernel
                dma_eng.dma_start(
                    out=y_d[i, 128 * o : 128 * (o + 1), r0:r1, :],
                    in_=osb[:, r0:r1, :],
                )


_CACHE: dict = {}


def _build():
    if "nc" in _CACHE:
        return _CACHE["nc"]
    nc = bacc.Bacc(
        "TRN2", target_bir_lowering=False, debug=False, num_devices=N_CORES
    )
    x_d = nc.dram_tensor("x", [NLOC, C, HW, HW], F32, kind="ExternalInput")
    g_d = nc.dram_tensor("gamma", [C, 1], F32, kind="ExternalInput")
    be_d = nc.dram_tensor("beta", [C, 1], F32, kind="ExternalInput")
    w_d = nc.dram_tensor("w", [C, C, 3, 3], F32, kind="ExternalInput")
    y_d = nc.dram_tensor("y", [NLOC, C, HW, HW], F32, kind="ExternalOutput")
    cc_in = nc.dram_tensor("cc_in", [2 * CB, 128], F32)
    cc_out = nc.dram_tensor(
        "cc_out", [N_CORES, 2 * CB, 128], F32, addr_space="Shared"
    )
    rsum_d = nc.dram_tensor("rsum_d", [2 * CB, 2 * CB], F32)

    from contextlib import ExitStack

    with tile.TileContext(nc) as tc, ExitStack() as ctx:
        _build_body(ctx, nc, tc, x_d, g_d, be_d, w_d, y_d, cc_in, cc_out, rsum_d)
    nc.compile()
    _CACHE["nc"] = nc
    return nc


def kernel(x, gamma, beta, W):
    x = np.ascontiguousarray(np.asarray(x, dtype=np.float32))
    gamma = np.ascontiguousarray(np.asarray(gamma, dtype=np.float32)).reshape(C, 1)
    beta = np.ascontiguousarray(np.asarray(beta, dtype=np.float32)).reshape(C, 1)
    W = np.ascontiguousarray(np.asarray(W, dtype=np.float32))
    nc = _build()
    in_maps = [
        {
            "x": x[NLOC * k : NLOC * (k + 1)],
            "gamma": gamma,
            "beta": beta,
            "w": W,
        }
        for k in range(N_CORES)
    ]
    res = run_bass_kernel_spmd(nc, in_maps, core_ids=list(range(N_CORES)))
    return np.concatenate(
        [res.results[k]["y"] for k in range(N_CORES)], axis=0
    )



# revision 31
# speedup vs baseline: 1.0398x; 1.0398x over previous
"""BinaryConv (BN + sign-binarize + 3x3 binary conv) on 8 Trainium2 NeuronCores.

Strategy (data-parallel over batch, per sharding hint):
  - Each of the 8 cores gets 4 of the 32 images; weights/gamma/beta replicated.
  - Per-core BN partial stats (mean, mean-square per channel) via bn_stats,
    AllGathered across cores (2KB payload, transposed stat-slot-major so the
    gathered blocks read back as 512B-contiguous runs), then the 8-rank sum +
    transpose back to channel-major happens in ONE fp32 PE matmul against a
    replicated identity: sync-BN exact.
  - Binarize via ScalarE Sign(gamma*x + (beta*std - mean*gamma)) — equivalent
    to the reference BN+sign since std>0 — into a zero-padded per-image
    layout (34-wide rows, both ci-halves stacked) in fp8 e4m3.
  - 3x3 conv = 9 shifted DoubleRow fp8 matmuls (contraction 256 in one pass)
    accumulated in PSUM. +/-1 inputs with fp32 PSUM accumulation are exact
    (integer sums), so the conv matches the fp32 reference bit-for-bit.
  - Weights: sign() on ScalarE, transposed to [ci][2][o] via PE transpose.
  - A dense fp8 warm-up matmul burst gated on the collective readback fills
    the stat-math latency window so the PE clock (HAM) is ramped when the
    conv burst starts; the last image's PSUM drain is split across
    vector/scalar (copies) and sync/gpsimd (stores) to shorten the tail.

Timing notes (measured): the collective cannot complete before ~70-90us from
execution start regardless of trigger time (per-execution CC arming under
this runtime; a lone 2KB AllGather takes the same). Everything before it
(x load, stats, weight prep) overlaps that wall; the post-collective path is
~46us: ~6us stats math + binarize lead-in, ~35us conv (at fp8 DoubleRow
peak), ~4.5us drain. remote_dma paths (SWDGE prepare/trigger, hostgen) hang
or fail to load under this runtime — a CC collective is the only working
cross-core channel.
"""

import numpy as np

import concourse.tile as tile
from concourse import bacc, mybir
from concourse.bass_utils import run_bass_kernel_spmd
from concourse.masks import make_identity

F32 = mybir.dt.float32
BF16 = mybir.dt.bfloat16
FP8 = mybir.dt.float8e4

N_CORES = 8
N = 32            # full batch
NLOC = N // N_CORES  # images per core
C = 256           # channels (in == out)
HW = 32           # spatial
CB = C // 128     # ci partition blocks
OB = C // 128     # o partition blocks
EPS = 1e-5

PADW = HW + 2     # padded row width
IMG_PAD = 1160    # per-image padded buffer (>= 34*34 + 2 margin, mult of 8)
# output row-chunks (r0, r1): each chunk's matmul free dim = (r1-r0)*34 <= 512
CHUNKS = [(0, 11), (11, 22), (22, 32)]
TAPS = [(dy, dx) for dy in range(3) for dx in range(3)]


def _wt_idx(t: int, b: int, o: int) -> int:
    return (t * CB + b) * OB + o


def _build_body(ctx, nc, tc, x_d, g_d, be_d, w_d, y_d, cc_in, cc_out, rsum_d):
    # ---------------- pools ----------------
    const = ctx.enter_context(tc.tile_pool(name="const", bufs=1))
    xin_p = ctx.enter_context(tc.tile_pool(name="xin", bufs=1))
    wpool = ctx.enter_context(tc.tile_pool(name="wpool", bufs=1))
    apool = ctx.enter_context(tc.tile_pool(name="apool", bufs=1))
    stat_p = ctx.enter_context(tc.tile_pool(name="stat", bufs=1))
    out_p = ctx.enter_context(tc.tile_pool(name="outp", bufs=1))
    ps_tr = ctx.enter_context(tc.tile_pool(name="pstr", bufs=2, space="PSUM"))
    ps_acc = ctx.enter_context(tc.tile_pool(name="psacc", bufs=1, space="PSUM"))

    # ---------------- load x (stats-critical) ----------------
    # 16 DMAs (one per ci-block x image x row-half) so all 16 HW DMA queues
    # stream in parallel; per-queue bandwidth was the previous limiter.
    xin = []
    x_dmas = []
    for b in range(CB):
        xb = xin_p.tile([128, NLOC, HW, HW], F32, name=f"xin{b}", tag=f"xin{b}")
        for i in range(NLOC):
            for h in range(2):
                eng = nc.sync if b == 0 else nc.scalar
                x_dmas.append(
                    eng.dma_start(
                        out=xb[:, i, 16 * h : 16 * (h + 1), :],
                        in_=x_d[i, 128 * b : 128 * (b + 1),
                                16 * h : 16 * (h + 1), :],
                    )
                )
        xin.append(xb)

    # ---------------- weight prep (independent of stats) ----------------
    ident = const.tile([128, 128], BF16, name="ident")
    make_identity(nc, ident[:])
    identf = const.tile([128, 128], F32, name="identf")
    make_identity(nc, identf[:])
    # replicated identity for the rank-sum matmul: rsum[(k, s), s'] = (s==s').
    # Built via DRAM bounce — compute engines can't write at partition
    # offset 4, DMAs can. All pre-wall.
    rsum = const.tile([N_CORES * 2 * CB, 2 * CB], F32, name="rsum")

    # ---------------- activation-table preload (scalar, off critical path) --
    # The post-collective math needs Sqrt/Copy tables; loading them here
    # (during the x transfer) removes the 1.3-1.5us ACT_TABLE_LOAD that
    # otherwise sits right after the collective readback.
    eps_t = const.tile([128, 1], F32, name="eps_t")
    nc.vector.memset(eps_t[:], EPS)
    dumm = const.tile([128, 1], F32, name="dumm")
    nc.scalar.activation(
        out=dumm[:], in_=eps_t[:],
        func=mybir.ActivationFunctionType.Sign,
    )
    nc.scalar.activation(
        out=dumm[:], in_=eps_t[:],
        func=mybir.ActivationFunctionType.Sqrt,
        bias=eps_t[:], scale=1.0,
    )
    nc.scalar.mul(out=dumm[:], in_=eps_t[:], mul=eps_t[:])
    nc.scalar.add(out=dumm[:], in_=dumm[:], add=eps_t[:])

    # junk rhs for the PE warm-up burst (contents irrelevant; memset only so
    # the race detector sees initialized data)
    dummy_a = const.tile([128, CB, 374], FP8, name="dummy_a")
    nc.gpsimd.memset(dummy_a[:], 0.0)


    # ---------------- zero only the padding of the activation buffers ------
    # (interior is fully overwritten by binarize; tiny strided memsets keep
    # both DVE and the collective-trigger path free)
    apad = [None] * NLOC
    for i in range(NLOC):
        ap = apool.tile([128, CB, IMG_PAD], FP8, name=f"apad{i}",
                        tag=f"apad{i}")
        nc.gpsimd.memset(ap[:, :, 0:35], 0.0)
        gaps = ap[:, :, 67 : 67 + 34 * HW].rearrange(
            "p b (h w) -> p b h w", w=PADW
        )[:, :, :, 0:2]
        nc.gpsimd.memset(gaps, 0.0)
        nc.gpsimd.memset(ap[:, :, 35 + 34 * HW - 2 : IMG_PAD], 0.0)
        apad[i] = ap

    # ---------------- local BN stats ----------------
    stats_rec = []
    for b in range(CB):
        xb = xin[b]
        rec = stat_p.tile([128, 2 * NLOC, 6], F32, name=f"rec{b}", tag=f"rec{b}")
        for i in range(NLOC):
            for h in range(2):
                nc.vector.bn_stats(
                    out=rec[:, 2 * i + h, :],
                    in_=xb[:, i, 16 * h : 16 * (h + 1), :].rearrange(
                        "p h w -> p (h w)"
                    ),
                )
        stats_rec.append(rec)

    # pack [mean_b, meansq_b] per ci-block into AllReduce payload
    arbuf = stat_p.tile([128, 2 * CB], F32, name="arbuf")
    tmp1 = stat_p.tile([128, 1], F32, name="tmp1")
    for b in range(CB):
        mv = stat_p.tile([128, 2], F32, name=f"mv{b}", tag=f"mv{b}")
        nc.vector.bn_aggr(out=mv[:], in_=stats_rec[b][:])
        nc.vector.tensor_copy(out=arbuf[:, 2 * b : 2 * b + 1], in_=mv[:, 0:1])
        nc.vector.tensor_mul(tmp1[:], mv[:, 0:1], mv[:, 0:1])
        nc.vector.tensor_add(arbuf[:, 2 * b + 1 : 2 * b + 2], mv[:, 1:2], tmp1[:])

    # Bounce the 2KB stats payload TRANSPOSED ([4, 128]: stat-slot major) so
    # each rank's gathered block is contiguous per stat-slot. The readback
    # then moves 512B-granule runs (32 descriptors vs 1024 for the
    # channel-major layout), and the 8-rank sum happens on the PE as an
    # accumulating-matmul transpose — no DVE tree-reduce on the critical
    # path. The transposing bounce write (4B granule, ~1us) is pre-wall.
    nc.gpsimd.dma_start(
        out=cc_in.rearrange("s c -> c s"), in_=arbuf[:]
    )
    # AllGather + local 8-way reduce instead of AllReduce: the gather is a
    # single mesh pass (~9.5us) vs ~12.7us for AllReduce.
    nc.gpsimd.collective_compute(
        "AllGather",
        mybir.AluOpType.bypass,
        replica_groups=[list(range(N_CORES))],
        ins=[cc_in.ap().opt()],
        outs=[cc_out.ap().opt()],
    )
    # ------- weight prep: W DMAs issued on the same HWDGE queues as x but
    # later in program order, so they start after the x issue stream drains —
    # no explicit gating needed. 4 DMAs split across both issue engines.
    wsign = []
    for o in range(OB):
        wraw = wpool.tile([128, C, 3, 3], F32, name=f"wraw{o}", tag=f"wraw{o}")
        for ch in range(2):
            (nc.sync if ch == 0 else nc.scalar).dma_start(
                out=wraw[:, 128 * ch : 128 * (ch + 1)],
                in_=w_d[128 * o : 128 * (o + 1),
                        128 * ch : 128 * (ch + 1), :, :],
            )
        ws = wpool.tile([128, C, 3, 3], BF16, name=f"wsign{o}", tag=f"wsign{o}")
        nc.scalar.activation(
            out=ws[:], in_=wraw[:], func=mybir.ActivationFunctionType.Sign
        )
        wsign.append(ws)

    # build rsum (queued after the x/W issue streams so it never delays them)
    nc.sync.dma_start(out=rsum_d[:, :], in_=identf[0 : 2 * CB, 0 : 2 * CB])
    for k in range(N_CORES):
        nc.scalar.dma_start(
            out=rsum[4 * k : 4 * (k + 1), :], in_=rsum_d[:, :]
        )

    # transposed binarized weights, DoubleRow layout:
    # wT[ci_local, tap*OB+o, ci_half, o_local]  (fp8)
    wT = wpool.tile([128, len(TAPS) * OB, CB, 128], FP8, name="wT")
    for t, (dy, dx) in enumerate(TAPS):
        for b in range(CB):
            for o in range(OB):
                ptr = ps_tr.tile([128, 128], BF16, name="ptr", tag="ptr", bufs=2)
                nc.tensor.transpose(
                    ptr[:], wsign[o][:, 128 * b : 128 * (b + 1), dy, dx], ident[:]
                )
                nc.vector.tensor_copy(
                    out=wT[:, t * OB + o, b, :], in_=ptr[:]
                )

    # readback all 8 ranks' transposed partials as one [32, 128] tile
    # ((rank, slot) on partitions; 512B-contiguous runs, 32 descriptors in a
    # single DMA), then ONE fp32 matmul against a replicated-identity [32, 4]
    # rhs sums the ranks AND restores channel-major in ~0.25us.
    gsall_t = stat_p.tile([N_CORES * 2 * CB, 128], F32, name="gsall_t")
    nc.sync.dma_start(
        out=gsall_t[:, :],
        in_=cc_out[:, :, :].rearrange("k s c -> (k s) c"),
    )
    gs_ps = ps_tr.tile([128, 2 * CB], F32, name="gs_ps", tag="ptr", bufs=2)
    nc.tensor.matmul(
        gs_ps[:], gsall_t[:, :], rsum[:, :], start=True, stop=True
    )
    # stat math reads the PSUM sum directly; the SBUF copy only feeds the PE
    # warm-up trigger (off the critical chain)
    gs = stat_p.tile([128, 2 * CB], F32, name="gs")
    nc.vector.tensor_copy(out=gs[:], in_=gs_ps[:])
    smean = gs_ps[:].rearrange("p (b s) -> p b s", s=2)[:, :, 0]  # [128, CB]
    smsq = gs_ps[:].rearrange("p (b s) -> p b s", s=2)[:, :, 1]

    # per-channel scale/shift computed as wide [128, CB] ops.
    # Since std > 0:  sign((x-mean)*gamma/std + beta)
    #              == sign(gamma*x + (beta*std - mean*gamma))
    # so scale = gamma (known before the AllReduce!) and
    # shift = beta*std - mean*gamma  (no reciprocal needed).
    gam = stat_p.tile([128, CB], F32, name="gam")
    bet = stat_p.tile([128, CB], F32, name="bet")
    for b in range(CB):
        nc.sync.dma_start(out=gam[:, b : b + 1], in_=g_d[128 * b : 128 * (b + 1), :])
        nc.sync.dma_start(out=bet[:, b : b + 1], in_=be_d[128 * b : 128 * (b + 1), :])
    inv = 1.0 / N_CORES
    # PE warm-up trigger: gpsimd (not ACT/DVE, which are on the critical
    # stat-math path) casts the summed stats into the warm-up rhs.
    junk = stat_p.tile([128, 4], BF16, name="junk")
    nc.gpsimd.tensor_copy(out=junk[:], in_=gs[:])
    mean_t = stat_p.tile([128, CB], F32, name="mean_t")
    nc.vector.tensor_scalar_mul(out=mean_t[:], in0=smean, scalar1=inv)
    msqr = stat_p.tile([128, CB], F32, name="msqr")
    nc.vector.tensor_mul(msqr[:], mean_t[:], mean_t[:])
    var_t = stat_p.tile([128, CB], F32, name="var_t")
    # var = (smsq * inv) - mean^2
    nc.vector.scalar_tensor_tensor(
        out=var_t[:],
        in0=smsq,
        scalar=inv,
        in1=msqr[:],
        op0=mybir.AluOpType.mult,
        op1=mybir.AluOpType.subtract,
    )
    # neg_mg = -(mean)*gamma, computed on DVE in parallel with var
    neg_mg = stat_p.tile([128, CB], F32, name="neg_mg")
    nc.vector.scalar_tensor_tensor(
        out=neg_mg[:],
        in0=smean,
        scalar=-inv,
        in1=gam[:],
        op0=mybir.AluOpType.mult,
        op1=mybir.AluOpType.mult,
    )
    # sqrt + shift entirely on ScalarE: binarize follows on the same queue
    # with no cross-engine hop
    std_t = stat_p.tile([128, CB], F32, name="std_t")
    nc.scalar.activation(
        out=std_t[:],
        in_=var_t[:],
        func=mybir.ActivationFunctionType.Sqrt,
        bias=eps_t[:],
        scale=1.0,
    )
    # shift = beta*std + neg_mg on DVE (idle here), freeing the scalar queue
    # so the binarize SIGNs start right after the sqrt
    sh_t = stat_p.tile([128, CB], F32, name="sh_t")
    nc.vector.tensor_mul(sh_t[:], std_t[:], bet[:])
    nc.vector.tensor_add(sh_t[:], sh_t[:], neg_mg[:])
    scale_t = [gam[:, b : b + 1] for b in range(CB)]
    shift_t = [sh_t[:, b : b + 1] for b in range(CB)]

    # warm-up matmuls (results discarded) — dense fp8 DoubleRow bursts sized
    # to fill the stat-math + binarize latency window. Tiny matmuls don't
    # trigger the HAM clock ramp (observed: ramp fires ~3.2us into a dense
    # burst); these do, so the conv starts at full PE clock.
    pw0 = ps_tr.tile([128, 4], F32, name="pw0", tag="ptr", bufs=2)
    nc.tensor.matmul(pw0[:], ident[:], junk[:], start=True, stop=True)
    for k in range(10):
        pw = ps_tr.tile([128, 374], F32, name="pw", tag="ptr", bufs=2)
        nc.tensor.matmul(
            pw[:],
            wT[:, 0, :, :],
            dummy_a[:, :, 0:374],
            start=True,
            stop=True,
            perf_mode=mybir.MatmulPerfMode.DoubleRow,
        )

    # ---------------- binarize into padded layout (fp8, DoubleRow pairs) ----
    # Row-thirds matched to the conv chunks' input needs (chunk0: rows 0-11,
    # chunk1: 10-22, chunk2: 21-31) so each chunk's matmuls unblock as soon
    # as its rows are written.
    BIN_SPANS = [(0, 12), (12, 23), (23, HW)]
    for i in range(NLOC):
        for r0s, r1s in BIN_SPANS:
            for b in range(CB):
                interior = apad[i][:, b, 35 : 35 + 34 * HW].rearrange(
                    "p (h w) -> p h w", w=PADW
                )[:, r0s:r1s, 0:HW]
                nc.scalar.activation(
                    out=interior,
                    in_=xin[b][:, i, r0s:r1s, :],
                    func=mybir.ActivationFunctionType.Sign,
                    scale=scale_t[b],
                    bias=shift_t[b],
                )

    # ---------------- conv: 9 shifted DoubleRow matmuls, PSUM accumulate ----
    for i in range(NLOC):
        psum = {}
        for o in range(OB):
            for ci, (r0, r1) in enumerate(CHUNKS):
                psum[(o, ci)] = ps_acc.tile(
                    [128, (r1 - r0) * PADW], F32, name=f"acc{o}_{ci}",
                    tag=f"acc{o}_{ci}", bufs=1,
                )
        for t, (dy, dx) in enumerate(TAPS):
            toff = dy * PADW + dx
            first = t == 0
            last = t == len(TAPS) - 1
            # o-major: the lhsT weight block stays stationary across the
            # three chunk matmuls (chunk-major would reload it every matmul)
            for o in range(OB):
                lhsT = wT[:, t * OB + o, :, :]
                for ci, (r0, r1) in enumerate(CHUNKS):
                    ncols = (r1 - r0) * PADW
                    off = r0 * PADW + toff
                    nc.tensor.matmul(
                        psum[(o, ci)][:],
                        lhsT,
                        apad[i][:, :, off : off + ncols],
                        start=first,
                        stop=last,
                        perf_mode=mybir.MatmulPerfMode.DoubleRow,
                    )
        last_img = i == NLOC - 1
        for o in range(OB):
            osb = out_p.tile([128, HW, HW], F32, name=f"osb{o}", tag=f"osb{o}",
                             bufs=2)
            for ci, (r0, r1) in enumerate(CHUNKS):
                src = psum[(o, ci)][:].rearrange("p (r c) -> p r c", c=PADW)[
                    :, :, 0:HW
                ]
                # last image: binarize is done, so scalar (copies) and gpsimd
                # (DMA issue) are free — split the drain across both pairs of
                # engines to halve the kernel tail.
                if last_img and (o * len(CHUNKS) + ci) % 2 == 1:
                    nc.scalar.activation(
                        out=osb[:, r0:r1, :], in_=src,
                        func=mybir.ActivationFunctionType.Copy,
                    )
                    dma_eng = nc.gpsimd
                else:
                    nc.vector.tensor_copy(out=osb[:, r0:r1, :], in_=src)
                    dma_eng = nc.sync
                # per-chunk output DMA so the store of the final chunks
                # overlaps the remaining matmuls instead of tailing the kernel
                dma_eng.dma_start(
                    out=y_d[i, 128 * o : 128 * (o + 1), r0:r1, :],
                    in_=osb[:, r0:r1, :],
                )


_CACHE: dict = {}


def _build():
    if "nc" in _CACHE:
        return _CACHE["nc"]
    nc = bacc.Bacc(
        "TRN2", target_bir_lowering=False, debug=False, num_devices=N_CORES
    )
    x_d = nc.dram_tensor("x", [NLOC, C, HW, HW], F32, kind="ExternalInput")
    g_d = nc.dram_tensor("gamma", [C, 1], F32, kind="ExternalInput")
    be_d = nc.dram_tensor("beta", [C, 1], F32, kind="ExternalInput")
    w_d = nc.dram_tensor("w", [C, C, 3, 3], F32, kind="ExternalInput")
    y_d = nc.dram_tensor("y", [NLOC, C, HW, HW], F32, kind="ExternalOutput")
    cc_in = nc.dram_tensor("cc_in", [2 * CB, 128], F32)
    cc_out = nc.dram_tensor(
        "cc_out", [N_CORES, 2 * CB, 128], F32, addr_space="Shared"
    )
    rsum_d = nc.dram_tensor("rsum_d", [2 * CB, 2 * CB], F32)

    from contextlib import ExitStack

    with tile.TileContext(nc) as tc, ExitStack() as ctx:
        _build_body(ctx, nc, tc, x_d, g_d, be_d, w_d, y_d, cc_in, cc_out, rsum_d)
    nc.compile()
    _CACHE["nc"] = nc
    return nc


def kernel(x, gamma, beta, W):
    x = np.ascontiguousarray(np.asarray(x, dtype=np.float32))
    gamma = np.ascontiguousarray(np.asarray(gamma, dtype=np.float32)).reshape(C, 1)
    beta = np.ascontiguousarray(np.asarray(beta, dtype=np.float32)).reshape(C, 1)
    W = np.ascontiguousarray(np.asarray(W, dtype=np.float32))
    nc = _build()
    in_maps = [
        {
            "x": x[NLOC * k : NLOC * (k + 1)],
            "gamma": gamma,
            "beta": beta,
            "w": W,
        }
        for k in range(N_CORES)
    ]
    res = run_bass_kernel_spmd(nc, in_maps, core_ids=list(range(N_CORES)))
    return np.concatenate(
        [res.results[k]["y"] for k in range(N_CORES)], axis=0
    )

